# revision 1
# baseline (speedup 1.0000x reference)
"""CfC head (mLSTM-style scan) Trainium2 kernel.

Math (per timestep t, per (b,h)):
    pre_g = xt*Wg_w + Wg_b            (xt = (x_codes-65)/100)
    i_t = exp(pre_i - n), f_t = exp(pre_f - n), o_t = exp(pre_o - n)
    g_t = sigmoid(pre_g); lam = sigmoid(pre_l)
    c   = f_t*c + i_t*g_t
    h   = (h + DT*o_t*sigmoid(c)) / (1 + DT*lam)
    n  += 0.01*(i_t + f_t + o_t - 3)
    y_t = h @ proj_w.T + proj_b

Device mapping: H=1024 sharded over 8 cores (128 h-values per core, one SBUF
partition each); free dim packs (batch-major, time-minor) blocks of TB steps.
The n-recurrence is handled per block by tracking the within-block drift
delta = n - n_blockstart, linearized as the affine scan
    delta_t = (1 - 0.01*P_t) * delta_{t-1} + (0.01*P_t - 0.03),
    P_t = (Ei+Ef+Eo)_t * exp(-n_blockstart),
which runs as one tensor_tensor_scan over the whole block (validated: rel err
1.4e-4 at TB=32 vs exact). c and h are exact affine scans given en = exp(-n):
    c_t = (Ef_t*en) * c_{t-1} + (Ei_t*G_t*en)
    h_t = L1_t * h_{t-1} + L1_t*DT*Eo_t*en*sigmoid(c_t),  L1 = 1/(1+DT*lam)
L1 uses the Neumann form 1 - q + q^2 = (q-0.5)^2 + 0.75 (q = DT*lam <= 0.01).
Sigmoids use tanh so every activation (exp/tanh/square) lives in the single
"exp_and_others" ACT table set (no table reloads).

Most tiles are fp16 (DVE 2x tensor_tensor / 4x tensor_scalar modes); Sq, L1
and h stay fp32 — L1 is the h-scan decay rate whose error is amplified by
1/(1-L1) ~ 200x, and h feeds the output directly (validated numerically:
rel err 4.3e-4 vs reference with this dtype split).

y partials (over each core's 128 h) are accumulated on PE into PSUM and
summed across cores on the host.
"""

import os
from contextlib import ExitStack

import numpy as np

import concourse.bacc as bacc
import concourse.mybir as mybir
import concourse.tile as tile
from concourse.bass_utils import run_bass_kernel_spmd

AF = mybir.ActivationFunctionType
OP = mybir.AluOpType
F32 = mybir.dt.float32
F16 = mybir.dt.float16

B, S, H = 64, 2048, 1024
NCORES = 8
HC = H // NCORES  # 128 h-values per core = partition dim
DT = 0.01

TB = int(os.environ.get("KERNEL_TB", "32"))  # timesteps per block
CCLAMP = 3.0e4  # c-carry clamp; sigmoid(c>=17) == 1.0f so this is exact

_cached = {}
_last_results = None


def build_program(s=S, tb=TB):
    nb = s // tb
    nfd = B * tb           # free dim of block tiles, (b-major, t-minor)
    nslab = nfd // 128     # 128-wide matmul slabs per block

    nc = bacc.Bacc(
        "TRN2", target_bir_lowering=False, debug=False, num_devices=NCORES
    )
    x_d = nc.dram_tensor("x", [B, s], F16, kind="ExternalInput").ap()
    wv_d = nc.dram_tensor("wv", [HC, 10], F32, kind="ExternalInput").ap()
    pj_d = nc.dram_tensor("projT", [HC, 2], F32, kind="ExternalInput").ap()
    n0_d = nc.dram_tensor("n0", [HC, 1], F32, kind="ExternalInput").ap()
    y_d = nc.dram_tensor("yout", [nb, 128, tb], F32, kind="ExternalOutput").ap()

    def r3(ap):  # [128, nfd] -> [128, B, tb]
        return ap.rearrange("p (b t) -> p b t", t=tb)

    with tile.TileContext(nc) as tc, ExitStack() as ctx:
        wp = ctx.enter_context(tc.tile_pool(name="w", bufs=1))
        pha = ctx.enter_context(tc.tile_pool(name="pha", bufs=2))
        chn = ctx.enter_context(tc.tile_pool(name="chn", bufs=1))
        sm = ctx.enter_context(tc.tile_pool(name="sm", bufs=2))
        pp = ctx.enter_context(tc.tile_pool(name="pp", bufs=2, space="PSUM"))

        wv = wp.tile([HC, 10], F32)
        nc.sync.dma_start(wv[:], wv_d)
        pj = wp.tile([HC, 2], F32)
        nc.sync.dma_start(pj[:], pj_d)
        n0t = wp.tile([HC, 1], F32)
        nc.sync.dma_start(n0t[:], n0_d)

        # carries: n at block start (per h,b), exp(-n), c, h
        Nc = wp.tile([HC, B], F32)
        nc.vector.memset(Nc[:], 0.0)
        nc.vector.tensor_scalar(Nc[:], Nc[:], n0t[:, 0:1], None, OP.add)
        ENc = wp.tile([HC, B], F16)
        nc.scalar.activation(ENc[:], Nc[:], AF.Exp, scale=-1.0)
        cz = wp.tile([HC, B], F16)
        nc.vector.memset(cz[:], 0.0)
        hz = wp.tile([HC, B], F32)
        nc.vector.memset(hz[:], 0.0)
        Cc_v, Hc_v = cz[:], hz[:]
        bqm = wp.tile([HC, 1], F32)
        nc.vector.memset(bqm[:], DT / 2 - 0.5)

        for k in range(nb):
            t0 = k * tb
            X = pha.tile([128, nfd], F16, tag="X")
            nc.sync.dma_start(
                r3(X[:]), x_d[:, t0 : t0 + tb].partition_broadcast(128)
            )
            # gate pre-activations, fused through ACT scale/bias
            Ei = pha.tile([128, nfd], F16, tag="Ei")
            nc.scalar.activation(
                Ei[:], X[:], AF.Exp, bias=wv[:, 1:2], scale=wv[:, 0:1]
            )
            Ef = pha.tile([128, nfd], F16, tag="Ef")
            nc.scalar.activation(
                Ef[:], X[:], AF.Exp, bias=wv[:, 3:4], scale=wv[:, 2:3]
            )
            Eo = pha.tile([128, nfd], F16, tag="Eo")
            nc.scalar.activation(
                Eo[:], X[:], AF.Exp, bias=wv[:, 5:6], scale=wv[:, 4:5]
            )
            Tg = pha.tile([128, nfd], F16, tag="Tg")
            nc.scalar.activation(
                Tg[:], X[:], AF.Tanh, bias=wv[:, 7:8], scale=wv[:, 6:7]
            )
            Tl = pha.tile([128, nfd], F16, tag="Tl")
            nc.scalar.activation(
                Tl[:], X[:], AF.Tanh, bias=wv[:, 9:10], scale=wv[:, 8:9]
            )

            # G = 0.5*Tg+0.5 ; EiG = Ei*G  (both land in Tg)
            nc.vector.tensor_scalar(Tg[:], Tg[:], 0.5, 0.5, OP.mult, OP.add)
            nc.vector.tensor_mul(Tg[:], Ei[:], Tg[:])
            # Esum = Ei+Ef+Eo, then P = Esum*exp(-Nc)  (lands in Ei)
            nc.vector.tensor_add(Ei[:], Ei[:], Ef[:])
            nc.vector.tensor_add(Ei[:], Ei[:], Eo[:])
            ENc_bc = ENc[:].unsqueeze(2).broadcast_to([HC, B, tb])
            nc.vector.tensor_mul(r3(Ei[:]), r3(Ei[:]), ENc_bc)

            # delta scan: delta = (1-0.01P)*prev + (0.01P-0.03)
            a = chn.tile([128, nfd], F16, tag="a")
            nc.vector.tensor_scalar(a[:], Ei[:], -0.01, 1.0, OP.mult, OP.add)
            rr = chn.tile([128, nfd], F16, tag="r")
            nc.vector.tensor_scalar(rr[:], Ei[:], 0.01, -0.03, OP.mult, OP.add)
            nc.vector.memset(r3(a[:])[:, :, 0], 0.0)
            d = chn.tile([128, nfd], F16, tag="d")
            nc.vector.tensor_tensor_scan(d[:], a[:], rr[:], 0.0, OP.mult, OP.add)
            nc.vector.tensor_add(Nc[:], Nc[:], r3(d[:])[:, :, tb - 1])

            # EN = exp(-(Nc_old + delta_{t-1})): shifted exp, slots = 1, * ENc
            ED = chn.tile([128, nfd], F16, tag="ED")
            nc.scalar.activation(ED[:, 1:nfd], d[:, 0 : nfd - 1], AF.Exp, scale=-1.0)
            nc.vector.memset(r3(ED[:])[:, :, 0], 1.0)
            nc.vector.tensor_mul(r3(ED[:]), r3(ED[:]), ENc_bc)

            # c scan: a_c = Ef*EN (in Ef), b_c = EiG*EN (in Tg)
            nc.vector.tensor_mul(Ef[:], Ef[:], ED[:])
            nc.vector.tensor_mul(Tg[:], Tg[:], ED[:])
            t64 = sm.tile([HC, B], F16, tag="t64")
            nc.vector.tensor_mul(t64[:], r3(Ef[:])[:, :, 0], Cc_v)
            nc.vector.tensor_add(
                r3(Tg[:])[:, :, 0], r3(Tg[:])[:, :, 0], t64[:]
            )
            nc.vector.memset(r3(Ef[:])[:, :, 0], 0.0)
            c = chn.tile([128, nfd], F16, tag="c")
            nc.vector.tensor_tensor_scan(c[:], Ef[:], Tg[:], 0.0, OP.mult, OP.add)

            # sigmoid(c) via tanh; L1 = 1-q+q^2 = (q-0.5)^2+0.75 with
            # q = DT*lam = DT/2*(Tl+1): fold q into the Square ACT directly:
            # Sq = (DT/2*Tl + (DT/2-0.5))^2
            Tc = chn.tile([128, nfd], F16, tag="Tc")
            nc.scalar.activation(Tc[:], c[:], AF.Tanh, scale=0.5)
            Sq = chn.tile([128, nfd], F32, tag="Sq")
            nc.scalar.activation(Sq[:], Tl[:], AF.Square, bias=bqm[:], scale=DT / 2)
            L1 = chn.tile([128, nfd], F32, tag="L1")
            nc.vector.tensor_scalar(L1[:], Sq[:], 0.75, None, OP.add)
            # L1D = DT/2 * L1 on the scalar engine (fp16 out)
            L1D = chn.tile([128, nfd], F16, tag="L1D")
            nc.scalar.mul(L1D[:], L1[:], DT / 2)

            # b_h = Eo*L1D*EN*(Tc+1)   (lands in Eo)
            nc.vector.tensor_mul(Eo[:], Eo[:], L1D[:])
            nc.vector.tensor_mul(Eo[:], Eo[:], ED[:])
            nc.vector.tensor_scalar(Tc[:], Tc[:], 1.0, None, OP.add)
            nc.vector.tensor_mul(Eo[:], Eo[:], Tc[:])
            t64b = sm.tile([HC, B], F32, tag="t64b")
            nc.vector.tensor_mul(t64b[:], r3(L1[:])[:, :, 0], Hc_v)
            nc.vector.tensor_add(
                r3(Eo[:])[:, :, 0], r3(Eo[:])[:, :, 0], t64b[:]
            )
            nc.vector.memset(r3(L1[:])[:, :, 0], 0.0)
            h = chn.tile([128, nfd], F32, tag="h")
            nc.vector.tensor_tensor_scan(h[:], L1[:], Eo[:], 0.0, OP.mult, OP.add)

            # y partials: psum[m, 2j:2j+2] = h-slab_j.T @ projT
            ps = pp.tile([128, tb], F32)
            for j in range(nslab):
                nc.tensor.matmul(
                    ps[:, 2 * j : 2 * j + 2],
                    h[:, 128 * j : 128 * (j + 1)],
                    pj[:],
                    start=True,
                    stop=True,
                )
            ysb = sm.tile([128, tb], F32, tag="ysb")
            nc.scalar.copy(ysb[:], ps[:])
            nc.sync.dma_start(y_d[k], ysb[:])

            # c grows without bound for lanes with persistent f_t > 1 (the
            # reference saturates through sigmoid(inf)=1).  Clamp the carry so
            # the next block's 0*carry segment reset never sees inf; any clamp
            # >= ~30 leaves sigmoid(c) exactly 1.0f.
            Ccl = sm.tile([HC, B], F16, tag="ccl")
            nc.vector.tensor_scalar_min(Ccl[:], r3(c[:])[:, :, tb - 1], CCLAMP)
            Cc_v = Ccl[:]
            Hc_v = r3(h[:])[:, :, tb - 1]
            nc.scalar.activation(ENc[:], Nc[:], AF.Exp, scale=-1.0)

    nc.compile()
    return nc


def _get_program():
    key = (S, TB)
    if key not in _cached:
        _cached[key] = build_program(S, TB)
    return _cached[key]


def host_inputs(x_codes, Wi_w, Wi_b, Wf_w, Wf_b, Wo_w, Wo_b, Wg_w, Wg_b,
                Wl_w, Wl_b, proj_w, proj_b, n_init):
    """Fold input normalization into per-gate ACT scale/bias; shard over H."""
    f = lambda v: np.asarray(v, np.float32)
    cols = []
    for (w, b) in ((Wi_w, Wi_b), (Wf_w, Wf_b), (Wo_w, Wo_b)):
        cols += [f(w) / 100.0, f(b) - 0.65 * f(w)]
    for (w, b) in ((Wg_w, Wg_b), (Wl_w, Wl_b)):
        cols += [f(w) / 200.0, (f(b) - 0.65 * f(w)) / 2.0]
    wv_full = np.stack(cols, axis=1).astype(np.float32)  # [H, 10]
    x = np.ascontiguousarray(f(x_codes)).astype(np.float16)
    pw = f(proj_w)
    n0 = f(n_init)
    maps = []
    for k in range(NCORES):
        hs = slice(k * HC, (k + 1) * HC)
        maps.append({
            "x": x,
            "wv": np.ascontiguousarray(wv_full[hs]),
            "projT": np.ascontiguousarray(pw[:, hs].T),
            "n0": np.ascontiguousarray(n0[hs].reshape(HC, 1)),
        })
    return maps


def assemble_output(results, proj_b, s=S, tb=TB):
    nb = s // tb
    nslab = (B * tb) // 128
    bper = 128 // tb  # batches per slab
    y = np.zeros((B, s, 2), np.float64)
    for k in range(NCORES):
        yc = np.asarray(results[k]["yout"], np.float64)
        ycr = yc.reshape(nb, bper, tb, nslab, 2)
        y += np.transpose(ycr, (3, 1, 0, 2, 4)).reshape(B, s, 2)
    y += np.asarray(proj_b, np.float64)[None, None, :]
    return y.astype(np.float32)


def kernel(**inputs):
    global _last_results
    nc = _get_program()
    maps = host_inputs(**inputs)
    res = run_bass_kernel_spmd(
        nc, maps, list(range(NCORES)),
        trace=bool(os.environ.get("KTRACE")),
        tmpdir=os.environ.get("KTRACE_DIR") or None,
    )
    _last_results = res
    return assemble_output(res.results, inputs["proj_b"])



# revision 4
# speedup vs baseline: 1.1838x; 1.1838x over previous
"""CfC head (mLSTM-style scan) Trainium2 kernel, v2.

Math (per timestep t, per (b,h)):
    pre_g = xt*Wg_w + Wg_b            (xt = (x_codes-65)/100)
    i_t = exp(pre_i - n), f_t = exp(pre_f - n), o_t = exp(pre_o - n)
    g_t = sigmoid(pre_g); lam = sigmoid(pre_l)
    c   = f_t*c + i_t*g_t
    h   = (h + DT*o_t*sigmoid(c)) / (1 + DT*lam)
    n  += 0.01*(i_t + f_t + o_t - 3)
    y_t = h @ proj_w.T + proj_b

Device mapping: H=1024 sharded over 8 cores (128 h-values per core = the
partition dim); free dim packs (batch-major, time-minor) blocks of TB steps.

Within a block the n-drift delta = n - n_blockstart follows the affine scan
    delta_t = (1 - 0.01*P_t)*delta_{t-1} + (0.01*P_t - 0.03),
    P_t = (Ei+Ef+Eo)_t * exp(-n0).
v2 runs the scan on dt = delta - 1 so the additive operand is the STATIC
tile -0.03 (only the per-segment first column is data-dependent):
    dt_t = a_t*dt_{t-1} - 0.03,  a_t = 1 - 0.01*P_t,  dt-seg0 = 0.01*P_0-1.03.
The gate correction uses e^{-delta} ~= 1 - delta = -dt (error delta^2/2,
|delta| <~ 0.1), so the per-step gate scale tensor is a single multiply:
    ENd_t = e^{-n0} * (1 - delta_{t-1}) = ENneg * dt_{t-1},  ENneg = -e^{-n0}
with no exp() on the ACT engine. c and h are exact affine scans:
    c_t = (Ef_t*ENd_t) * c_{t-1} + (Ei_t*g_t*ENd_t)
    h_t = L1_t * h_{t-1} + L1_t*DT*Eo_t*ENd_t*sigmoid(c_t),  L1 = 1/(1+DT*lam)
L1 uses the Neumann form 1 - q + q^2 = (q-0.5)^2 + 0.75 (q = DT*lam <= 0.01)
computed as Sq on ACT (fp32) and L1 = Sq + 0.75 on ACT; the h-input factor
L1D = DT/2*L1 is a DVE tensor_scalar from Sq. Sigmoids use tanh so all ACT
functions (exp/tanh/square/identity) share one table set.

Esum = Ei+Ef+Eo runs on the PE as identity-matmul PSUM accumulation; the
a-coefficient is one scalar_tensor_tensor from PSUM. Carries track
EN32 = exp(-n) (fp32) directly: EN32 *= exp(-delta_end) per block - n itself
is never materialized. Tiles are fp16 except dt/Sq/L1/EN32 (fp32); L1 is the
h-scan decay whose error is amplified ~200x so it must stay fp32.

y partials (over each core's 128 h) are accumulated on PE into PSUM with the
h-slab as the stationary operand ([128,2] PSUM out per slab -> cheap 128-lane
ACT copy), and summed across cores on the host.
"""

import os
from contextlib import ExitStack

import numpy as np

import concourse.bacc as bacc
import concourse.mybir as mybir
import concourse.tile as tile
from concourse.bass_utils import run_bass_kernel_spmd

AF = mybir.ActivationFunctionType
OP = mybir.AluOpType
F32 = mybir.dt.float32
F16 = mybir.dt.float16

B, S, H = 64, 2048, 1024
NCORES = 8
HC = H // NCORES  # 128 h-values per core = partition dim
DT = 0.01

TB = int(os.environ.get("KERNEL_TB", "32"))  # timesteps per block
CCLAMP = 3.0e4  # c-carry clamp; sigmoid(c>=17) == 1.0f so this is exact

_cached = {}
_last_results = None


def build_program(s=S, tb=TB):
    nb = s // tb
    nfd = B * tb           # free dim of block tiles, (b-major, t-minor)
    nslab = nfd // 128     # 128-wide matmul slabs per block

    nc = bacc.Bacc(
        "TRN2", target_bir_lowering=False, debug=False, num_devices=NCORES
    )
    x_d = nc.dram_tensor("x", [B, s], F16, kind="ExternalInput").ap()
    wv_d = nc.dram_tensor("wv", [HC, 10], F32, kind="ExternalInput").ap()
    pj_d = nc.dram_tensor("projT", [HC, 2], F16, kind="ExternalInput").ap()
    en0_d = nc.dram_tensor("en0", [HC, 1], F32, kind="ExternalInput").ap()
    id_d = nc.dram_tensor("ident", [128, 128], F16, kind="ExternalInput").ap()
    y_d = nc.dram_tensor("yout", [nb, 128, tb], F32, kind="ExternalOutput").ap()

    def r3(ap):  # [128, nfd] -> [128, B, tb]
        return ap.rearrange("p (b t) -> p b t", t=tb)

    with tile.TileContext(nc) as tc, ExitStack() as ctx:
        wp = ctx.enter_context(tc.tile_pool(name="w", bufs=1))
        pha = ctx.enter_context(tc.tile_pool(name="pha", bufs=2))
        chn = ctx.enter_context(tc.tile_pool(name="chn", bufs=1))
        sm = ctx.enter_context(tc.tile_pool(name="sm", bufs=2))
        pp = ctx.enter_context(tc.tile_pool(name="pp", bufs=2, space="PSUM"))
        pe = ctx.enter_context(tc.tile_pool(name="pe", bufs=1, space="PSUM"))

        wv = wp.tile([HC, 10], F32)
        nc.sync.dma_start(wv[:], wv_d)
        pj = wp.tile([HC, 2], F16)
        nc.sync.dma_start(pj[:], pj_d)
        en0t = wp.tile([HC, 1], F32)
        nc.sync.dma_start(en0t[:], en0_d)
        ident = wp.tile([128, 128], F16)
        nc.sync.dma_start(ident[:], id_d)

        # static additive operand of the dt-scan: -0.03 everywhere; col 0 of
        # each b-segment is rewritten per block.
        rst = wp.tile([HC, nfd], F16)
        nc.vector.memset(rst[:], -0.03)
        bqm = wp.tile([HC, 1], F32)
        nc.vector.memset(bqm[:], DT / 2 - 0.5)
        b75 = wp.tile([HC, 1], F32)
        nc.vector.memset(b75[:], 0.75)
        bm1 = wp.tile([HC, 1], F32)
        nc.vector.memset(bm1[:], -1.0)

        # carries: EN32 = exp(-n) fp32; fp16 +/- copies; c and h carries
        EN32 = wp.tile([HC, B], F32)
        nc.vector.memset(EN32[:], 1.0)
        nc.vector.tensor_scalar(EN32[:], EN32[:], en0t[:, 0:1], None, OP.mult)
        ENp = wp.tile([HC, B], F16)   # +exp(-n0)
        nc.vector.tensor_copy(ENp[:], EN32[:])
        ENn = wp.tile([HC, B], F16)   # -exp(-n0)
        nc.vector.tensor_scalar(ENn[:], EN32[:], -1.0, None, OP.mult)
        cz = wp.tile([HC, B], F16)
        nc.vector.memset(cz[:], 0.0)
        hz = wp.tile([HC, B], F16)
        nc.vector.memset(hz[:], 0.0)
        Cc_v, Hc_v = cz[:], hz[:]

        # stage A tiles for block 0
        def stage_a(k):
            """X-load, 5 gate ACTs, Sq/L1, PE Esum for block k."""
            X = pha.tile([128, nfd], F16, tag="X")
            nc.sync.dma_start(
                r3(X[:]), x_d[:, k * tb : (k + 1) * tb].partition_broadcast(128)
            )
            Ei = pha.tile([128, nfd], F16, tag="Ei")
            nc.scalar.activation(
                Ei[:], X[:], AF.Exp, bias=wv[:, 1:2], scale=wv[:, 0:1]
            )
            Ef = pha.tile([128, nfd], F16, tag="Ef")
            nc.scalar.activation(
                Ef[:], X[:], AF.Exp, bias=wv[:, 3:4], scale=wv[:, 2:3]
            )
            Eo = pha.tile([128, nfd], F16, tag="Eo")
            nc.scalar.activation(
                Eo[:], X[:], AF.Exp, bias=wv[:, 5:6], scale=wv[:, 4:5]
            )
            Tg = pha.tile([128, nfd], F16, tag="Tg")
            nc.scalar.activation(
                Tg[:], X[:], AF.Tanh, bias=wv[:, 7:8], scale=wv[:, 6:7]
            )
            Tl = pha.tile([128, nfd], F16, tag="Tl")
            nc.scalar.activation(
                Tl[:], X[:], AF.Tanh, bias=wv[:, 9:10], scale=wv[:, 8:9]
            )
            # Sq = (DT/2*Tl + (DT/2-0.5))^2 ; L1 = Sq + 0.75  (both fp32, ACT)
            Sq = pha.tile([128, nfd], F32, tag="Sq")
            nc.scalar.activation(
                Sq[:], Tl[:], AF.Square, bias=bqm[:], scale=DT / 2
            )
            L1 = pha.tile([128, nfd], F32, tag="L1")
            nc.scalar.activation(L1[:], Sq[:], AF.Identity, bias=b75[:])
            # Esum into PSUM via identity-matmul accumulation
            ps_es = pe.tile([128, nfd], F32, tag="es")
            for c4 in range(nfd // 512):
                sl = slice(512 * c4, 512 * (c4 + 1))
                nc.tensor.matmul(
                    ps_es[:, sl], ident[:], Ei[:, sl], start=True, stop=False
                )
                nc.tensor.matmul(
                    ps_es[:, sl], ident[:], Ef[:, sl], start=False, stop=False
                )
                nc.tensor.matmul(
                    ps_es[:, sl], ident[:], Eo[:, sl], start=False, stop=True
                )
            return X, Ei, Ef, Eo, Tg, Tl, Sq, L1, ps_es

    # ---- main loop with 1-block software pipeline ----
        tiles = stage_a(0)
        for k in range(nb):
            X, Ei, Ef, Eo, Tg, Tl, Sq, L1, ps_es = tiles
            if k + 1 < nb:
                tiles = stage_a(k + 1)

            ENp_v, ENn_v = ENp[:], ENn[:]
            ENp_bc = ENp_v.unsqueeze(2).broadcast_to([HC, B, tb])
            ENn_bc = ENn_v.unsqueeze(2).broadcast_to([HC, B, tb])

            # a = 1 - 0.01*Esum*exp(-n0): STT from PSUM then +1
            a = chn.tile([128, nfd], F16, tag="a")
            nc.vector.scalar_tensor_tensor(
                r3(a[:]), r3(ps_es[:]), -0.01, ENp_bc, OP.mult, OP.mult
            )
            nc.vector.tensor_scalar(a[:], a[:], 1.0, None, OP.add)
            # dt-scan: static rst except col0 = 0.01*P_0 - 1.03 = -a_0 - 0.03
            nc.vector.tensor_scalar(
                r3(rst[:])[:, :, 0], r3(a[:])[:, :, 0], -1.0, -0.03,
                OP.mult, OP.add,
            )
            nc.vector.memset(r3(a[:])[:, :, 0], 0.0)
            dt = chn.tile([128, nfd], F32, tag="dt")
            nc.vector.tensor_tensor_scan(
                dt[:], a[:], rst[:], 0.0, OP.mult, OP.add
            )

            # EN carry update: EN32 *= exp(-delta_end) = exp(-dt_end - 1)
            Eend = sm.tile([HC, B], F32, tag="Eend")
            nc.scalar.activation(
                Eend[:], r3(dt[:])[:, :, tb - 1], AF.Exp, bias=bm1[:], scale=-1.0
            )
            nc.vector.tensor_mul(EN32[:], EN32[:], Eend[:])
            nc.vector.tensor_copy(ENp[:], EN32[:])
            nc.vector.tensor_scalar(ENn[:], EN32[:], -1.0, None, OP.mult)

            # ENd_t = -e^{-n0} * dt_{t-1}  (= e^{-n0}(1-delta_{t-1}))
            ENd = chn.tile([128, nfd], F16, tag="ENd")
            nc.vector.tensor_mul(
                r3(ENd[:])[:, :, 1:tb],
                ENn_v.unsqueeze(2).broadcast_to([HC, B, tb - 1]),
                r3(dt[:])[:, :, 0 : tb - 1],
            )
            nc.vector.tensor_copy(r3(ENd[:])[:, :, 0], ENp_v)

            # c-scan operands: fc = Ef*ENd (in Ef), ic = Ei*ENd*g (in Ei)
            nc.vector.tensor_mul(Ef[:], Ef[:], ENd[:])
            nc.vector.tensor_mul(Ei[:], Ei[:], ENd[:])
            nc.vector.tensor_scalar(Tg[:], Tg[:], 0.5, 0.5, OP.mult, OP.add)
            nc.vector.tensor_mul(Ei[:], Ei[:], Tg[:])
            t64 = sm.tile([HC, B], F16, tag="t64")
            nc.vector.tensor_mul(t64[:], r3(Ef[:])[:, :, 0], Cc_v)
            nc.vector.tensor_add(
                r3(Ei[:])[:, :, 0], r3(Ei[:])[:, :, 0], t64[:]
            )
            nc.vector.memset(r3(Ef[:])[:, :, 0], 0.0)
            c = chn.tile([128, nfd], F16, tag="c")
            nc.vector.tensor_tensor_scan(c[:], Ef[:], Ei[:], 0.0, OP.mult, OP.add)

            Ccl = sm.tile([HC, B], F16, tag="ccl")
            nc.vector.tensor_scalar_min(Ccl[:], r3(c[:])[:, :, tb - 1], CCLAMP)

            # sigmoid(c) via tanh (same ACT table as exp)
            Tc = chn.tile([128, nfd], F16, tag="Tc")
            nc.scalar.activation(Tc[:], c[:], AF.Tanh, scale=0.5)

            # h input: bh = Eo*ENd*L1D*(Tc+1), L1D = DT/2*Sq + 0.75*DT/2
            nc.vector.tensor_mul(Eo[:], Eo[:], ENd[:])
            L1D = chn.tile([128, nfd], F16, tag="L1D")
            nc.vector.tensor_scalar(
                L1D[:], Sq[:], DT / 2, 0.75 * DT / 2, OP.mult, OP.add
            )
            nc.vector.tensor_mul(Eo[:], Eo[:], L1D[:])
            nc.vector.tensor_scalar(Tc[:], Tc[:], 1.0, None, OP.add)
            nc.vector.tensor_mul(Eo[:], Eo[:], Tc[:])
            t64b = sm.tile([HC, B], F32, tag="t64b")
            nc.vector.tensor_mul(t64b[:], r3(L1[:])[:, :, 0], Hc_v)
            nc.vector.tensor_add(
                r3(Eo[:])[:, :, 0], r3(Eo[:])[:, :, 0], t64b[:]
            )
            nc.vector.memset(r3(L1[:])[:, :, 0], 0.0)
            h = chn.tile([128, nfd], F16, tag="h")
            nc.vector.tensor_tensor_scan(h[:], L1[:], Eo[:], 0.0, OP.mult, OP.add)

            # y partials: psum[m, 2j:2j+2] = h-slab_j.T @ projT
            ps = pp.tile([128, tb], F32, tag="y")
            for j in range(nslab):
                nc.tensor.matmul(
                    ps[:, 2 * j : 2 * j + 2],
                    h[:, 128 * j : 128 * (j + 1)],
                    pj[:],
                    start=True,
                    stop=True,
                )
            ysb = sm.tile([128, tb], F32, tag="ysb")
            nc.scalar.copy(ysb[:], ps[:])
            nc.sync.dma_start(y_d[k], ysb[:])

            Cc_v = Ccl[:]
            Hc_v = r3(h[:])[:, :, tb - 1]

    nc.compile()
    return nc


def _get_program():
    key = (S, TB)
    if key not in _cached:
        _cached[key] = build_program(S, TB)
    return _cached[key]


def host_inputs(x_codes, Wi_w, Wi_b, Wf_w, Wf_b, Wo_w, Wo_b, Wg_w, Wg_b,
                Wl_w, Wl_b, proj_w, proj_b, n_init):
    """Fold input normalization into per-gate ACT scale/bias; shard over H."""
    f = lambda v: np.asarray(v, np.float32)
    cols = []
    for (w, b) in ((Wi_w, Wi_b), (Wf_w, Wf_b), (Wo_w, Wo_b)):
        cols += [f(w) / 100.0, f(b) - 0.65 * f(w)]
    for (w, b) in ((Wg_w, Wg_b), (Wl_w, Wl_b)):
        cols += [f(w) / 200.0, (f(b) - 0.65 * f(w)) / 2.0]
    wv_full = np.stack(cols, axis=1).astype(np.float32)  # [H, 10]
    x = np.ascontiguousarray(f(x_codes)).astype(np.float16)
    pw = f(proj_w)
    en0 = np.exp(-f(n_init))
    ident = np.eye(128, dtype=np.float16)
    maps = []
    for k in range(NCORES):
        hs = slice(k * HC, (k + 1) * HC)
        maps.append({
            "x": x,
            "wv": np.ascontiguousarray(wv_full[hs]),
            "projT": np.ascontiguousarray(pw[:, hs].T.astype(np.float16)),
            "en0": np.ascontiguousarray(en0[hs].reshape(HC, 1)),
            "ident": ident,
        })
    return maps


def assemble_output(results, proj_b, s=S, tb=TB):
    nb = s // tb
    nslab = (B * tb) // 128
    bper = 128 // tb  # batches per slab
    y = np.zeros((B, s, 2), np.float64)
    for k in range(NCORES):
        yc = np.asarray(results[k]["yout"], np.float64)
        ycr = yc.reshape(nb, bper, tb, nslab, 2)
        y += np.transpose(ycr, (3, 1, 0, 2, 4)).reshape(B, s, 2)
    y += np.asarray(proj_b, np.float64)[None, None, :]
    return y.astype(np.float32)


def kernel(**inputs):
    global _last_results
    nc = _get_program()
    maps = host_inputs(**inputs)
    res = run_bass_kernel_spmd(
        nc, maps, list(range(NCORES)),
        trace=bool(os.environ.get("KTRACE")),
        tmpdir=os.environ.get("KTRACE_DIR") or None,
    )
    _last_results = res
    return assemble_output(res.results, inputs["proj_b"])


# revision 5
# speedup vs baseline: 1.5792x; 1.3340x over previous
"""CfC head (mLSTM-style scan) Trainium2 kernel, v2.

Math (per timestep t, per (b,h)):
    pre_g = xt*Wg_w + Wg_b            (xt = (x_codes-65)/100)
    i_t = exp(pre_i - n), f_t = exp(pre_f - n), o_t = exp(pre_o - n)
    g_t = sigmoid(pre_g); lam = sigmoid(pre_l)
    c   = f_t*c + i_t*g_t
    h   = (h + DT*o_t*sigmoid(c)) / (1 + DT*lam)
    n  += 0.01*(i_t + f_t + o_t - 3)
    y_t = h @ proj_w.T + proj_b

Device mapping: H=1024 sharded over 8 cores (128 h-values per core = the
partition dim); free dim packs (batch-major, time-minor) blocks of TB steps.

Within a block the n-drift delta = n - n_blockstart follows the affine scan
    delta_t = (1 - 0.01*P_t)*delta_{t-1} + (0.01*P_t - 0.03),
    P_t = (Ei+Ef+Eo)_t * exp(-n0).
v2 runs the scan on dt = delta - 1 so the additive operand is the STATIC
tile -0.03 (only the per-segment first column is data-dependent):
    dt_t = a_t*dt_{t-1} - 0.03,  a_t = 1 - 0.01*P_t,  dt-seg0 = 0.01*P_0-1.03.
The gate correction uses e^{-delta} ~= 1 - delta = -dt (error delta^2/2,
|delta| <~ 0.1), so the per-step gate scale tensor is a single multiply:
    ENd_t = e^{-n0} * (1 - delta_{t-1}) = ENneg * dt_{t-1},  ENneg = -e^{-n0}
with no exp() on the ACT engine. c and h are exact affine scans:
    c_t = (Ef_t*ENd_t) * c_{t-1} + (Ei_t*g_t*ENd_t)
    h_t = L1_t * h_{t-1} + L1_t*DT*Eo_t*ENd_t*sigmoid(c_t),  L1 = 1/(1+DT*lam)
L1 uses the Neumann form 1 - q + q^2 = (q-0.5)^2 + 0.75 (q = DT*lam <= 0.01)
computed as Sq on ACT (fp32) and L1 = Sq + 0.75 on ACT; the h-input factor
L1D = DT/2*L1 is a DVE tensor_scalar from Sq. Sigmoids use tanh so all ACT
functions (exp/tanh/square/identity) share one table set.

Esum = Ei+Ef+Eo runs on the PE as identity-matmul PSUM accumulation; the
a-coefficient is one scalar_tensor_tensor from PSUM. Carries track
EN32 = exp(-n) (fp32) directly: EN32 *= exp(-delta_end) per block - n itself
is never materialized. Tiles are fp16 except dt/Sq/L1/EN32 (fp32); L1 is the
h-scan decay whose error is amplified ~200x so it must stay fp32.

y partials (over each core's 128 h) are accumulated on PE into PSUM with the
h-slab as the stationary operand ([128,2] PSUM out per slab -> cheap 128-lane
ACT copy), and summed across cores on the host.
"""

import os
from contextlib import ExitStack

import numpy as np

import concourse.bacc as bacc
import concourse.mybir as mybir
import concourse.tile as tile
from concourse.bass_utils import run_bass_kernel_spmd

AF = mybir.ActivationFunctionType
OP = mybir.AluOpType
F32 = mybir.dt.float32
F16 = mybir.dt.float16

B, S, H = 64, 2048, 1024
NCORES = 8
HC = H // NCORES  # 128 h-values per core = partition dim
DT = 0.01

TB = int(os.environ.get("KERNEL_TB", "32"))  # timesteps per block
CCLAMP = 3.0e4  # c-carry clamp; sigmoid(c>=17) == 1.0f so this is exact

_cached = {}
_last_results = None


def build_program(s=S, tb=TB):
    nb = s // tb
    nfd = B * tb           # free dim of block tiles, (b-major, t-minor)
    nslab = nfd // 128     # 128-wide matmul slabs per block

    nc = bacc.Bacc(
        "TRN2", target_bir_lowering=False, debug=False, num_devices=NCORES
    )
    x_d = nc.dram_tensor("x", [B, s], F16, kind="ExternalInput").ap()
    wv_d = nc.dram_tensor("wv", [HC, 10], F32, kind="ExternalInput").ap()
    pj_d = nc.dram_tensor("projT", [HC, 2], F16, kind="ExternalInput").ap()
    en0_d = nc.dram_tensor("en0", [HC, 1], F32, kind="ExternalInput").ap()
    id_d = nc.dram_tensor("ident", [128, 128], F16, kind="ExternalInput").ap()
    y_d = nc.dram_tensor("yout", [nb, 128, tb], F32, kind="ExternalOutput").ap()

    def r3(ap):  # [128, nfd] -> [128, B, tb]
        return ap.rearrange("p (b t) -> p b t", t=tb)

    with tile.TileContext(nc) as tc, ExitStack() as ctx:
        wp = ctx.enter_context(tc.tile_pool(name="w", bufs=1))
        pha = ctx.enter_context(tc.tile_pool(name="pha", bufs=2))
        chn = ctx.enter_context(tc.tile_pool(name="chn", bufs=1))
        sm = ctx.enter_context(tc.tile_pool(name="sm", bufs=2))
        pp = ctx.enter_context(tc.tile_pool(name="pp", bufs=2, space="PSUM"))
        pe = ctx.enter_context(tc.tile_pool(name="pe", bufs=1, space="PSUM"))

        wv = wp.tile([HC, 10], F32)
        nc.sync.dma_start(wv[:], wv_d)
        pj = wp.tile([HC, 2], F16)
        nc.sync.dma_start(pj[:], pj_d)
        en0t = wp.tile([HC, 1], F32)
        nc.sync.dma_start(en0t[:], en0_d)
        ident = wp.tile([128, 128], F16)
        nc.sync.dma_start(ident[:], id_d)

        # static additive operand of the dt-scan: -0.03 everywhere; col 0 of
        # each b-segment is rewritten per block.
        rst = wp.tile([HC, nfd], F16)
        nc.vector.memset(rst[:], -0.03)
        bqm = wp.tile([HC, 1], F32)
        nc.vector.memset(bqm[:], DT / 2 - 0.5)
        b75 = wp.tile([HC, 1], F32)
        nc.vector.memset(b75[:], 0.75)
        bm1 = wp.tile([HC, 1], F32)
        nc.vector.memset(bm1[:], -1.0)

        # carries: EN32 = exp(-n) fp32; fp16 +/- copies; c and h carries
        EN32 = wp.tile([HC, B], F32)
        nc.vector.memset(EN32[:], 1.0)
        nc.vector.tensor_scalar(EN32[:], EN32[:], en0t[:, 0:1], None, OP.mult)
        ENp = wp.tile([HC, B], F16)   # +exp(-n0)
        nc.vector.tensor_copy(ENp[:], EN32[:])
        ENn = wp.tile([HC, B], F16)   # -exp(-n0)
        nc.vector.tensor_scalar(ENn[:], EN32[:], -1.0, None, OP.mult)
        cz = wp.tile([HC, B], F16)
        nc.vector.memset(cz[:], 0.0)
        hz = wp.tile([HC, B], F16)
        nc.vector.memset(hz[:], 0.0)
        Cc_v, Hc_v = cz[:], hz[:]

        ones1 = wp.tile([1, 128], F16)
        nc.vector.memset(ones1[:], 1.0)

        def stage_x(k):
            """x staging DMA + PE broadcast into the shared XE PSUM tile."""
            xs = pha.tile([1, nfd], F16, tag="xs")
            nc.sync.dma_start(
                xs[:].rearrange("p (b t) -> p b t", t=tb),
                x_d[:, k * tb : (k + 1) * tb].unsqueeze(0),
            )
            ps_xe = pe.tile([128, nfd], F32, tag="xe")
            for c4 in range(nfd // 512):
                sl = slice(512 * c4, 512 * (c4 + 1))
                nc.tensor.matmul(
                    ps_xe[:, sl], ones1[:], xs[:, sl], start=True, stop=True
                )
            return ps_xe

        def stage_gates1(k, ps_xe):
            """first gate batch: 3 exps + Tg from PSUM X."""
            Ei = pha.tile([128, nfd], F16, tag="Ei")
            nc.scalar.activation(
                Ei[:], ps_xe[:], AF.Exp, bias=wv[:, 1:2], scale=wv[:, 0:1]
            )
            Ef = pha.tile([128, nfd], F16, tag="Ef")
            nc.scalar.activation(
                Ef[:], ps_xe[:], AF.Exp, bias=wv[:, 3:4], scale=wv[:, 2:3]
            )
            Eo = pha.tile([128, nfd], F16, tag="Eo")
            nc.scalar.activation(
                Eo[:], ps_xe[:], AF.Exp, bias=wv[:, 5:6], scale=wv[:, 4:5]
            )
            Tg = pha.tile([128, nfd], F16, tag="Tg")
            nc.scalar.activation(
                Tg[:], ps_xe[:], AF.Tanh, bias=wv[:, 7:8], scale=wv[:, 6:7]
            )
            return Ei, Ef, Eo, Tg

        def stage_gates2(k, ps_xe):
            """second gate batch: Tl, Sq, L1."""
            Tl = pha.tile([128, nfd], F16, tag="Tl")
            nc.scalar.activation(
                Tl[:], ps_xe[:], AF.Tanh, bias=wv[:, 9:10], scale=wv[:, 8:9]
            )
            Sq = pha.tile([128, nfd], F32, tag="Sq")
            nc.scalar.activation(
                Sq[:], Tl[:], AF.Square, bias=bqm[:], scale=DT / 2
            )
            return Tl, Sq

        def stage_esum(k, ps_xe, Ei, Ef, Eo):
            """Esum accumulates into the XE banks after the gates read X."""
            for c4 in range(nfd // 512):
                sl = slice(512 * c4, 512 * (c4 + 1))
                nc.tensor.matmul(
                    ps_xe[:, sl], ident[:], Ei[:, sl], start=True, stop=False
                )
                nc.tensor.matmul(
                    ps_xe[:, sl], ident[:], Ef[:, sl], start=False, stop=False
                )
                nc.tensor.matmul(
                    ps_xe[:, sl], ident[:], Eo[:, sl], start=False, stop=True
                )
            return ps_xe

        def stage_l1(k, Sq):
            L1 = pha.tile([128, nfd], F32, tag="L1")
            nc.scalar.activation(L1[:], Sq[:], AF.Identity, bias=b75[:])
            return L1

    # ---- main loop with software pipeline ----
        ps_xe0 = stage_x(0)
        g1 = stage_gates1(0, ps_xe0)
        g2 = stage_gates2(0, ps_xe0)
        es0 = stage_esum(0, ps_xe0, g1[0], g1[1], g1[2])
        l10 = stage_l1(0, g2[1])
        tiles = (g1[0], g1[1], g1[2], g1[3], g2[0], g2[1], l10, es0)
        nxt = {}
        for k in range(nb):
            Ei, Ef, Eo, Tg, Tl, Sq, L1, ps_es = tiles
            have_next = k + 1 < nb
            if have_next:
                nxt_xe = stage_x(k + 1)
                nxt_g1 = stage_gates1(k + 1, nxt_xe)

            ENp_v, ENn_v = ENp[:], ENn[:]
            ENp_bc = ENp_v.unsqueeze(2).broadcast_to([HC, B, tb])
            ENn_bc = ENn_v.unsqueeze(2).broadcast_to([HC, B, tb])

            # a = 1 - 0.01*Esum*exp(-n0): STT from PSUM then +1
            a = chn.tile([128, nfd], F16, tag="a")
            nc.vector.scalar_tensor_tensor(
                r3(a[:]), r3(ps_es[:]), -0.01, ENp_bc, OP.mult, OP.mult
            )
            nc.vector.tensor_scalar(a[:], a[:], 1.0, None, OP.add)
            # dt-scan: static rst except col0 = 0.01*P_0 - 1.03 = -a_0 - 0.03
            nc.vector.tensor_scalar(
                r3(rst[:])[:, :, 0], r3(a[:])[:, :, 0], -1.0, -0.03,
                OP.mult, OP.add,
            )
            nc.vector.memset(r3(a[:])[:, :, 0], 0.0)
            dt = chn.tile([128, nfd], F32, tag="dt")
            nc.vector.tensor_tensor_scan(
                dt[:], a[:], rst[:], 0.0, OP.mult, OP.add
            )

            # ENd_t = -e^{-n0} * dt_{t-1}  (= e^{-n0}(1-delta_{t-1}))
            # (must read the PRE-update ENn/ENp of this block)
            Eend = sm.tile([HC, B], F32, tag="Eend")
            nc.scalar.activation(
                Eend[:], r3(dt[:])[:, :, tb - 1], AF.Exp, bias=bm1[:], scale=-1.0
            )
            ENd = chn.tile([128, nfd], F16, tag="ENd")
            nc.vector.tensor_mul(
                r3(ENd[:])[:, :, 1:tb],
                ENn_v.unsqueeze(2).broadcast_to([HC, B, tb - 1]),
                r3(dt[:])[:, :, 0 : tb - 1],
            )
            nc.vector.tensor_copy(r3(ENd[:])[:, :, 0], ENp_v)
            # EN carry update: EN32 *= exp(-delta_end) = exp(-dt_end - 1)
            nc.vector.tensor_mul(EN32[:], EN32[:], Eend[:])
            nc.vector.tensor_copy(ENp[:], EN32[:])
            nc.vector.tensor_scalar(ENn[:], EN32[:], -1.0, None, OP.mult)

            if have_next:
                nxt_g2 = stage_gates2(k + 1, nxt_xe)

            # c-scan operands: fc = Ef*ENd (in Ef), ic = Ei*ENd*g (in Ei)
            nc.vector.tensor_mul(Ef[:], Ef[:], ENd[:])
            nc.vector.tensor_mul(Ei[:], Ei[:], ENd[:])
            nc.vector.tensor_scalar(Tg[:], Tg[:], 0.5, 0.5, OP.mult, OP.add)
            nc.vector.tensor_mul(Ei[:], Ei[:], Tg[:])
            t64 = sm.tile([HC, B], F16, tag="t64")
            nc.vector.tensor_mul(t64[:], r3(Ef[:])[:, :, 0], Cc_v)
            nc.vector.tensor_add(
                r3(Ei[:])[:, :, 0], r3(Ei[:])[:, :, 0], t64[:]
            )
            nc.vector.memset(r3(Ef[:])[:, :, 0], 0.0)
            c = chn.tile([128, nfd], F16, tag="c")
            nc.vector.tensor_tensor_scan(c[:], Ef[:], Ei[:], 0.0, OP.mult, OP.add)

            Ccl = sm.tile([HC, B], F16, tag="ccl")
            nc.vector.tensor_scalar_min(Ccl[:], r3(c[:])[:, :, tb - 1], CCLAMP)

            if have_next:
                nxt_es = stage_esum(k + 1, nxt_xe, nxt_g1[0], nxt_g1[1], nxt_g1[2])
                nxt_l1 = stage_l1(k + 1, nxt_g2[1])

            # sigmoid(c) via tanh (same ACT table as exp)
            Tc = chn.tile([128, nfd], F16, tag="Tc")
            nc.scalar.activation(Tc[:], c[:], AF.Tanh, scale=0.5)

            # h input: bh = Eo*ENd*L1D*(Tc+1), L1D = DT/2*Sq + 0.75*DT/2
            nc.vector.tensor_mul(Eo[:], Eo[:], ENd[:])
            L1D = chn.tile([128, nfd], F16, tag="L1D")
            nc.vector.tensor_scalar(
                L1D[:], Sq[:], DT / 2, 0.75 * DT / 2, OP.mult, OP.add
            )
            nc.vector.tensor_mul(Eo[:], Eo[:], L1D[:])
            nc.vector.tensor_scalar(Tc[:], Tc[:], 1.0, None, OP.add)
            nc.vector.tensor_mul(Eo[:], Eo[:], Tc[:])
            t64b = sm.tile([HC, B], F32, tag="t64b")
            nc.vector.tensor_mul(t64b[:], r3(L1[:])[:, :, 0], Hc_v)
            nc.vector.tensor_add(
                r3(Eo[:])[:, :, 0], r3(Eo[:])[:, :, 0], t64b[:]
            )
            nc.vector.memset(r3(L1[:])[:, :, 0], 0.0)
            h = chn.tile([128, nfd], F16, tag="h")
            nc.vector.tensor_tensor_scan(h[:], L1[:], Eo[:], 0.0, OP.mult, OP.add)

            # y partials: psum[m, 2j:2j+2] = h-slab_j.T @ projT
            ps = pp.tile([128, tb], F32, tag="y")
            for j in range(nslab):
                nc.tensor.matmul(
                    ps[:, 2 * j : 2 * j + 2],
                    h[:, 128 * j : 128 * (j + 1)],
                    pj[:],
                    start=True,
                    stop=True,
                )
            ysb = sm.tile([128, tb], F32, tag="ysb")
            nc.scalar.copy(ysb[:], ps[:])
            nc.sync.dma_start(y_d[k], ysb[:])

            Cc_v = Ccl[:]
            Hc_v = r3(h[:])[:, :, tb - 1]
            if have_next:
                tiles = (nxt_g1[0], nxt_g1[1], nxt_g1[2], nxt_g1[3],
                         nxt_g2[0], nxt_g2[1], nxt_l1, nxt_es)

    nc.compile()
    return nc


def _get_program():
    key = (S, TB)
    if key not in _cached:
        _cached[key] = build_program(S, TB)
    return _cached[key]


def host_inputs(x_codes, Wi_w, Wi_b, Wf_w, Wf_b, Wo_w, Wo_b, Wg_w, Wg_b,
                Wl_w, Wl_b, proj_w, proj_b, n_init):
    """Fold input normalization into per-gate ACT scale/bias; shard over H."""
    f = lambda v: np.asarray(v, np.float32)
    cols = []
    for (w, b) in ((Wi_w, Wi_b), (Wf_w, Wf_b), (Wo_w, Wo_b)):
        cols += [f(w) / 100.0, f(b) - 0.65 * f(w)]
    for (w, b) in ((Wg_w, Wg_b), (Wl_w, Wl_b)):
        cols += [f(w) / 200.0, (f(b) - 0.65 * f(w)) / 2.0]
    wv_full = np.stack(cols, axis=1).astype(np.float32)  # [H, 10]
    x = np.ascontiguousarray(f(x_codes)).astype(np.float16)
    pw = f(proj_w)
    en0 = np.exp(-f(n_init))
    ident = np.eye(128, dtype=np.float16)
    maps = []
    for k in range(NCORES):
        hs = slice(k * HC, (k + 1) * HC)
        maps.append({
            "x": x,
            "wv": np.ascontiguousarray(wv_full[hs]),
            "projT": np.ascontiguousarray(pw[:, hs].T.astype(np.float16)),
            "en0": np.ascontiguousarray(en0[hs].reshape(HC, 1)),
            "ident": ident,
        })
    return maps


def assemble_output(results, proj_b, s=S, tb=TB):
    nb = s // tb
    nslab = (B * tb) // 128
    bper = 128 // tb  # batches per slab
    y = np.zeros((B, s, 2), np.float64)
    for k in range(NCORES):
        yc = np.asarray(results[k]["yout"], np.float64)
        ycr = yc.reshape(nb, bper, tb, nslab, 2)
        y += np.transpose(ycr, (3, 1, 0, 2, 4)).reshape(B, s, 2)
    y += np.asarray(proj_b, np.float64)[None, None, :]
    return y.astype(np.float32)


def kernel(**inputs):
    global _last_results
    nc = _get_program()
    maps = host_inputs(**inputs)
    res = run_bass_kernel_spmd(
        nc, maps, list(range(NCORES)),
        trace=bool(os.environ.get("KTRACE")),
        tmpdir=os.environ.get("KTRACE_DIR") or None,
    )
    _last_results = res
    return assemble_output(res.results, inputs["proj_b"])


# revision 7
# speedup vs baseline: 1.6161x; 1.0234x over previous
"""CfC head (mLSTM-style scan) Trainium2 kernel, v2.

Math (per timestep t, per (b,h)):
    pre_g = xt*Wg_w + Wg_b            (xt = (x_codes-65)/100)
    i_t = exp(pre_i - n), f_t = exp(pre_f - n), o_t = exp(pre_o - n)
    g_t = sigmoid(pre_g); lam = sigmoid(pre_l)
    c   = f_t*c + i_t*g_t
    h   = (h + DT*o_t*sigmoid(c)) / (1 + DT*lam)
    n  += 0.01*(i_t + f_t + o_t - 3)
    y_t = h @ proj_w.T + proj_b

Device mapping: H=1024 sharded over 8 cores (128 h-values per core = the
partition dim); free dim packs (batch-major, time-minor) blocks of TB steps.

Within a block the n-drift delta = n - n_blockstart follows the affine scan
    delta_t = (1 - 0.01*P_t)*delta_{t-1} + (0.01*P_t - 0.03),
    P_t = (Ei+Ef+Eo)_t * exp(-n0).
v2 runs the scan on dt = delta - 1 so the additive operand is the STATIC
tile -0.03 (only the per-segment first column is data-dependent):
    dt_t = a_t*dt_{t-1} - 0.03,  a_t = 1 - 0.01*P_t,  dt-seg0 = 0.01*P_0-1.03.
The gate correction uses e^{-delta} ~= 1 - delta = -dt (error delta^2/2,
|delta| <~ 0.1), so the per-step gate scale tensor is a single multiply:
    ENd_t = e^{-n0} * (1 - delta_{t-1}) = ENneg * dt_{t-1},  ENneg = -e^{-n0}
with no exp() on the ACT engine. c and h are exact affine scans:
    c_t = (Ef_t*ENd_t) * c_{t-1} + (Ei_t*g_t*ENd_t)
    h_t = L1_t * h_{t-1} + L1_t*DT*Eo_t*ENd_t*sigmoid(c_t),  L1 = 1/(1+DT*lam)
L1 uses the Neumann form 1 - q + q^2 = (q-0.5)^2 + 0.75 (q = DT*lam <= 0.01)
computed as Sq on ACT (fp32) and L1 = Sq + 0.75 on ACT; the h-input factor
L1D = DT/2*L1 is a DVE tensor_scalar from Sq. Sigmoids use tanh so all ACT
functions (exp/tanh/square/identity) share one table set.

Esum = Ei+Ef+Eo runs on the PE as identity-matmul PSUM accumulation; the
a-coefficient is one scalar_tensor_tensor from PSUM. Carries track
EN32 = exp(-n) (fp32) directly: EN32 *= exp(-delta_end) per block - n itself
is never materialized. Tiles are fp16 except dt/Sq/L1/EN32 (fp32); L1 is the
h-scan decay whose error is amplified ~200x so it must stay fp32.

y partials (over each core's 128 h) are accumulated on PE into PSUM with the
h-slab as the stationary operand ([128,2] PSUM out per slab -> cheap 128-lane
ACT copy), and summed across cores on the host.
"""

import os
from contextlib import ExitStack

import numpy as np

import concourse.bacc as bacc
import concourse.mybir as mybir
import concourse.tile as tile
from concourse.bass_utils import run_bass_kernel_spmd

AF = mybir.ActivationFunctionType
OP = mybir.AluOpType
F32 = mybir.dt.float32
F16 = mybir.dt.float16

B, S, H = 64, 2048, 1024
NCORES = 8
HC = H // NCORES  # 128 h-values per core = partition dim
DT = 0.01

TB = int(os.environ.get("KERNEL_TB", "32"))  # timesteps per block
CCLAMP = 3.0e4  # c-carry clamp; sigmoid(c>=17) == 1.0f so this is exact

_cached = {}
_last_results = None


def build_program(s=S, tb=TB):
    nb = s // tb
    nfd = B * tb           # free dim of block tiles, (b-major, t-minor)
    nslab = nfd // 128     # 128-wide matmul slabs per block

    nc = bacc.Bacc(
        "TRN2", target_bir_lowering=False, debug=False, num_devices=NCORES
    )
    x_d = nc.dram_tensor("x", [B, s], F16, kind="ExternalInput").ap()
    wv_d = nc.dram_tensor("wv", [HC, 10], F32, kind="ExternalInput").ap()
    pj_d = nc.dram_tensor("projT", [HC, 2], F16, kind="ExternalInput").ap()
    en0_d = nc.dram_tensor("en0", [HC, 1], F32, kind="ExternalInput").ap()
    id_d = nc.dram_tensor("ident", [128, 128], F16, kind="ExternalInput").ap()
    y_d = nc.dram_tensor("yout", [nb, 128, tb], F32, kind="ExternalOutput").ap()

    def r3(ap):  # [128, nfd] -> [128, B, tb]
        return ap.rearrange("p (b t) -> p b t", t=tb)

    with tile.TileContext(nc) as tc, ExitStack() as ctx:
        wp = ctx.enter_context(tc.tile_pool(name="w", bufs=1))
        pha = ctx.enter_context(tc.tile_pool(name="pha", bufs=2))
        chn = ctx.enter_context(tc.tile_pool(name="chn", bufs=1))
        sm = ctx.enter_context(tc.tile_pool(name="sm", bufs=2))
        pp = ctx.enter_context(tc.tile_pool(name="pp", bufs=2, space="PSUM"))
        pe = ctx.enter_context(tc.tile_pool(name="pe", bufs=1, space="PSUM"))

        wv = wp.tile([HC, 10], F32)
        nc.sync.dma_start(wv[:], wv_d)
        pj = wp.tile([HC, 2], F16)
        nc.sync.dma_start(pj[:], pj_d)
        en0t = wp.tile([HC, 1], F32)
        nc.sync.dma_start(en0t[:], en0_d)
        ident = wp.tile([128, 128], F16)
        nc.sync.dma_start(ident[:], id_d)

        # static additive operand of the dt-scan: -0.03 everywhere; col 0 of
        # each b-segment is rewritten per block.
        rst = wp.tile([HC, nfd], F16)
        nc.vector.memset(rst[:], -0.03)
        bqm = wp.tile([HC, 1], F32)
        nc.vector.memset(bqm[:], DT / 2 - 0.5)
        b75 = wp.tile([HC, 1], F32)
        nc.vector.memset(b75[:], 0.75)
        bm1 = wp.tile([HC, 1], F32)
        nc.vector.memset(bm1[:], -1.0)
        b05 = wp.tile([HC, 1], F32)
        nc.vector.memset(b05[:], 0.5)
        bp1 = wp.tile([HC, 1], F32)
        nc.vector.memset(bp1[:], 1.0)

        # carries: EN32 = exp(-n) fp32; fp16 +/- copies; c and h carries
        EN32 = wp.tile([HC, B], F32)
        nc.vector.memset(EN32[:], 1.0)
        nc.vector.tensor_scalar(EN32[:], EN32[:], en0t[:, 0:1], None, OP.mult)
        ENp = wp.tile([HC, B], F16)   # +exp(-n0)
        nc.vector.tensor_copy(ENp[:], EN32[:])
        ENn = wp.tile([HC, B], F16)   # -exp(-n0)
        nc.vector.tensor_scalar(ENn[:], EN32[:], -1.0, None, OP.mult)
        cz = wp.tile([HC, B], F16)
        nc.vector.memset(cz[:], 0.0)
        hz = wp.tile([HC, B], F16)
        nc.vector.memset(hz[:], 0.0)
        Cc_v, Hc_v = cz[:], hz[:]

        ones1 = wp.tile([1, 128], F16)
        nc.vector.memset(ones1[:], 1.0)

        def stage_x(k):
            """x staging DMA + PE broadcast into the shared XE PSUM tile."""
            xs = pha.tile([1, nfd], F16, tag="xs")
            nc.sync.dma_start(
                xs[:].rearrange("p (b t) -> p b t", t=tb),
                x_d[:, k * tb : (k + 1) * tb].unsqueeze(0),
            )
            ps_xe = pe.tile([128, nfd], F32, tag="xe")
            for c4 in range(nfd // 512):
                sl = slice(512 * c4, 512 * (c4 + 1))
                nc.tensor.matmul(
                    ps_xe[:, sl], ones1[:], xs[:, sl], start=True, stop=True
                )
            return ps_xe

        def stage_gates1(k, ps_xe):
            """first gate batch: 3 exps + Tg from PSUM X."""
            Ei = pha.tile([128, nfd], F16, tag="Ei")
            nc.scalar.activation(
                Ei[:], ps_xe[:], AF.Exp, bias=wv[:, 1:2], scale=wv[:, 0:1]
            )
            Ef = pha.tile([128, nfd], F16, tag="Ef")
            nc.scalar.activation(
                Ef[:], ps_xe[:], AF.Exp, bias=wv[:, 3:4], scale=wv[:, 2:3]
            )
            Eo = pha.tile([128, nfd], F16, tag="Eo")
            nc.scalar.activation(
                Eo[:], ps_xe[:], AF.Exp, bias=wv[:, 5:6], scale=wv[:, 4:5]
            )
            Tg = pha.tile([128, nfd], F16, tag="Tg")
            nc.scalar.activation(
                Tg[:], ps_xe[:], AF.Tanh, bias=wv[:, 7:8], scale=wv[:, 6:7]
            )
            return Ei, Ef, Eo, Tg

        def stage_gates2(k, ps_xe):
            """second gate batch: Tl, Sq, L1."""
            Tl = pha.tile([128, nfd], F16, tag="Tl")
            nc.scalar.activation(
                Tl[:], ps_xe[:], AF.Tanh, bias=wv[:, 9:10], scale=wv[:, 8:9]
            )
            Sq = pha.tile([128, nfd], F32, tag="Sq")
            nc.scalar.activation(
                Sq[:], Tl[:], AF.Square, bias=bqm[:], scale=DT / 2
            )
            return Tl, Sq

        def stage_esum(k, ps_xe, Ei, Ef, Eo):
            """Esum accumulates into the XE banks after the gates read X."""
            for c4 in range(nfd // 512):
                sl = slice(512 * c4, 512 * (c4 + 1))
                nc.tensor.matmul(
                    ps_xe[:, sl], ident[:], Ei[:, sl], start=True, stop=False
                )
                nc.tensor.matmul(
                    ps_xe[:, sl], ident[:], Ef[:, sl], start=False, stop=False
                )
                nc.tensor.matmul(
                    ps_xe[:, sl], ident[:], Eo[:, sl], start=False, stop=True
                )
            return ps_xe

        def stage_l1(k, Sq):
            L1 = pha.tile([128, nfd], F32, tag="L1")
            nc.scalar.activation(L1[:], Sq[:], AF.Identity, bias=b75[:])
            return L1

    # ---- main loop with software pipeline ----
        ps_xe0 = stage_x(0)
        g1 = stage_gates1(0, ps_xe0)
        g2 = stage_gates2(0, ps_xe0)
        es0 = stage_esum(0, ps_xe0, g1[0], g1[1], g1[2])
        l10 = stage_l1(0, g2[1])
        tiles = (g1[0], g1[1], g1[2], g1[3], g2[0], g2[1], l10, es0)
        nxt = {}
        for k in range(nb):
            Ei, Ef, Eo, Tg, Tl, Sq, L1, ps_es = tiles
            have_next = k + 1 < nb
            if have_next:
                nxt_xe = stage_x(k + 1)
                nxt_g1 = stage_gates1(k + 1, nxt_xe)

            ENp_v, ENn_v = ENp[:], ENn[:]
            ENp_bc = ENp_v.unsqueeze(2).broadcast_to([HC, B, tb])
            ENn_bc = ENn_v.unsqueeze(2).broadcast_to([HC, B, tb])

            # a = 1 - 0.01*Esum*exp(-n0): STT from PSUM then +1
            a = chn.tile([128, nfd], F16, tag="a")
            nc.vector.scalar_tensor_tensor(
                r3(a[:]), r3(ps_es[:]), -0.01, ENp_bc, OP.mult, OP.mult
            )
            nc.vector.tensor_scalar(a[:], a[:], 1.0, None, OP.add)
            # dt-scan: static rst except col0 = 0.01*P_0 - 1.03 = -a_0 - 0.03
            nc.vector.tensor_scalar(
                r3(rst[:])[:, :, 0], r3(a[:])[:, :, 0], -1.0, -0.03,
                OP.mult, OP.add,
            )
            nc.vector.memset(r3(a[:])[:, :, 0], 0.0)
            dt = chn.tile([128, nfd], F16, tag="dt")
            nc.vector.tensor_tensor_scan(
                dt[:], a[:], rst[:], 0.0, OP.mult, OP.add
            )

            # ENd_t = -e^{-n0} * dt_{t-1}  (= e^{-n0}(1-delta_{t-1}))
            # (must read the PRE-update ENn/ENp of this block)
            Eend = sm.tile([HC, B], F32, tag="Eend")
            nc.scalar.activation(
                Eend[:], r3(dt[:])[:, :, tb - 1], AF.Exp, bias=bm1[:], scale=-1.0
            )
            ENd = chn.tile([128, nfd], F16, tag="ENd")
            nc.vector.tensor_mul(
                r3(ENd[:])[:, :, 1:tb],
                ENn_v.unsqueeze(2).broadcast_to([HC, B, tb - 1]),
                r3(dt[:])[:, :, 0 : tb - 1],
            )
            nc.vector.tensor_copy(r3(ENd[:])[:, :, 0], ENp_v)
            # EN carry update: EN32 *= exp(-delta_end) = exp(-dt_end - 1)
            nc.vector.tensor_mul(EN32[:], EN32[:], Eend[:])
            nc.scalar.copy(ENp[:], EN32[:])
            nc.scalar.mul(ENn[:], EN32[:], -1.0)

            if have_next:
                nxt_g2 = stage_gates2(k + 1, nxt_xe)

            # c-scan operands: fc = Ef*ENd (in Ef), ic = Ei*ENd*g (in Ei)
            nc.scalar.activation(Tg[:], Tg[:], AF.Identity, bias=b05[:], scale=0.5)
            nc.vector.tensor_mul(Ef[:], Ef[:], ENd[:])
            nc.vector.tensor_mul(Ei[:], Ei[:], ENd[:])
            nc.vector.tensor_mul(Ei[:], Ei[:], Tg[:])
            t64 = sm.tile([HC, B], F16, tag="t64")
            nc.vector.tensor_mul(t64[:], r3(Ef[:])[:, :, 0], Cc_v)
            nc.vector.tensor_add(
                r3(Ei[:])[:, :, 0], r3(Ei[:])[:, :, 0], t64[:]
            )
            nc.vector.memset(r3(Ef[:])[:, :, 0], 0.0)
            c = chn.tile([128, nfd], F16, tag="c")
            nc.vector.tensor_tensor_scan(c[:], Ef[:], Ei[:], 0.0, OP.mult, OP.add)

            Ccl = sm.tile([HC, B], F16, tag="ccl")
            nc.vector.tensor_scalar_min(Ccl[:], r3(c[:])[:, :, tb - 1], CCLAMP)

            if have_next:
                nxt_es = stage_esum(k + 1, nxt_xe, nxt_g1[0], nxt_g1[1], nxt_g1[2])
                nxt_l1 = stage_l1(k + 1, nxt_g2[1])

            # sigmoid(c) via tanh (same ACT table as exp); Tc1 = Tc + 1 on ACT
            Tc = chn.tile([128, nfd], F16, tag="Tc")
            nc.scalar.activation(Tc[:], c[:], AF.Tanh, scale=0.5)
            nc.scalar.activation(Tc[:], Tc[:], AF.Identity, bias=bp1[:])

            # h input: bh = Eo*ENd*L1D*(Tc+1), L1D = DT/2*Sq + 0.75*DT/2
            nc.vector.tensor_mul(Eo[:], Eo[:], ENd[:])
            L1D = chn.tile([128, nfd], F16, tag="L1D")
            nc.vector.tensor_scalar(
                L1D[:], Sq[:], DT / 2, 0.75 * DT / 2, OP.mult, OP.add
            )
            nc.vector.tensor_mul(Eo[:], Eo[:], L1D[:])
            nc.vector.tensor_mul(Eo[:], Eo[:], Tc[:])
            t64b = sm.tile([HC, B], F32, tag="t64b")
            nc.vector.tensor_mul(t64b[:], r3(L1[:])[:, :, 0], Hc_v)
            nc.vector.tensor_add(
                r3(Eo[:])[:, :, 0], r3(Eo[:])[:, :, 0], t64b[:]
            )
            nc.vector.memset(r3(L1[:])[:, :, 0], 0.0)
            h = chn.tile([128, nfd], F16, tag="h")
            nc.vector.tensor_tensor_scan(h[:], L1[:], Eo[:], 0.0, OP.mult, OP.add)

            # y partials: psum[m, 2j:2j+2] = h-slab_j.T @ projT
            ps = pp.tile([128, tb], F32, tag="y")
            for j in range(nslab):
                nc.tensor.matmul(
                    ps[:, 2 * j : 2 * j + 2],
                    h[:, 128 * j : 128 * (j + 1)],
                    pj[:],
                    start=True,
                    stop=True,
                )
            ysb = sm.tile([128, tb], F32, tag="ysb")
            nc.scalar.copy(ysb[:], ps[:])
            nc.sync.dma_start(y_d[k], ysb[:])

            Cc_v = Ccl[:]
            Hc_v = r3(h[:])[:, :, tb - 1]
            if have_next:
                tiles = (nxt_g1[0], nxt_g1[1], nxt_g1[2], nxt_g1[3],
                         nxt_g2[0], nxt_g2[1], nxt_l1, nxt_es)

    nc.compile()
    return nc


def _get_program():
    key = (S, TB)
    if key not in _cached:
        _cached[key] = build_program(S, TB)
    return _cached[key]


def host_inputs(x_codes, Wi_w, Wi_b, Wf_w, Wf_b, Wo_w, Wo_b, Wg_w, Wg_b,
                Wl_w, Wl_b, proj_w, proj_b, n_init):
    """Fold input normalization into per-gate ACT scale/bias; shard over H."""
    f = lambda v: np.asarray(v, np.float32)
    cols = []
    for (w, b) in ((Wi_w, Wi_b), (Wf_w, Wf_b), (Wo_w, Wo_b)):
        cols += [f(w) / 100.0, f(b) - 0.65 * f(w)]
    for (w, b) in ((Wg_w, Wg_b), (Wl_w, Wl_b)):
        cols += [f(w) / 200.0, (f(b) - 0.65 * f(w)) / 2.0]
    wv_full = np.stack(cols, axis=1).astype(np.float32)  # [H, 10]
    x = np.ascontiguousarray(f(x_codes)).astype(np.float16)
    pw = f(proj_w)
    en0 = np.exp(-f(n_init))
    ident = np.eye(128, dtype=np.float16)
    maps = []
    for k in range(NCORES):
        hs = slice(k * HC, (k + 1) * HC)
        maps.append({
            "x": x,
            "wv": np.ascontiguousarray(wv_full[hs]),
            "projT": np.ascontiguousarray(pw[:, hs].T.astype(np.float16)),
            "en0": np.ascontiguousarray(en0[hs].reshape(HC, 1)),
            "ident": ident,
        })
    return maps


def assemble_output(results, proj_b, s=S, tb=TB):
    nb = s // tb
    nslab = (B * tb) // 128
    bper = 128 // tb  # batches per slab
    y = np.zeros((B, s, 2), np.float64)
    for k in range(NCORES):
        yc = np.asarray(results[k]["yout"], np.float64)
        ycr = yc.reshape(nb, bper, tb, nslab, 2)
        y += np.transpose(ycr, (3, 1, 0, 2, 4)).reshape(B, s, 2)
    y += np.asarray(proj_b, np.float64)[None, None, :]
    return y.astype(np.float32)


def kernel(**inputs):
    global _last_results
    nc = _get_program()
    maps = host_inputs(**inputs)
    res = run_bass_kernel_spmd(
        nc, maps, list(range(NCORES)),
        trace=bool(os.environ.get("KTRACE")),
        tmpdir=os.environ.get("KTRACE_DIR") or None,
    )
    _last_results = res
    return assemble_output(res.results, inputs["proj_b"])
